# revision 21
# baseline (speedup 1.0000x reference)
"""Multi-head causal attention (B=4, S=2048, D=1024, H=16) on 8 Trainium2 cores.

Strategy: tensor-parallel over heads (2 heads/core).
 - Host feeds each core xT = x^T [D, B*S] (fp32r + bf16 copies) plus that
   core's slice of w_in columns (q cols pre-scaled by 1/sqrt(dh)), and full
   w_out.
 - Phase 1: k,v chains in fp32r (output precision), q chain in bf16;
   qT/kT resident in SBUF; vT staged + PE-transposed to v-natural bf16 tiles
   with a ones column appended (flash-attention sum trick).
 - Phase 2 (bf16 matmuls): per (batch, head): scoresT[k,q] pairs of k-tiles
   into one 2-bank PSUM tile, one Exp per pair (ACT), causal mask multiply,
   ctxT[dh+1, q] accumulated on PE with v_aug stationary; row dh = sum(exp).
   Software-pipelined (next pair's scores before this pair's AV matmuls).
   Normalize via partition_broadcast + reciprocal_approx_fast.
 - Four AllToAlls (one per batch) reshard ctxT from head-split to row-split;
   all overlap attention/out-proj compute.
 - Phase 3: out rows-slice = ctxT_full^T @ w_out + b_out (fp32r).
Outputs per core: kT/vT head slices and out rows-slices; host reassembles.
"""

import numpy as np
import ml_dtypes
from contextlib import ExitStack

NCORES = 8
DH = 64
H = 16
HPC = H // NCORES          # heads per core = 2
D = H * DH                 # 1024
NKD = D // 128             # 8 contraction tiles over D
RC = 512                   # phase-1 row chunk
QC = 512                   # phase-2 query chunk

_CACHE = {}


def _build(B=4, S=2048):
    import concourse.tile as tile
    from concourse import bacc, mybir
    from concourse.masks import make_identity

    R = B * S
    W = S // NCORES        # per-batch A2A shard width (rows)
    NQC = S // QC
    f32 = mybir.dt.float32
    f32r = mybir.dt.float32r
    bf16 = mybir.dt.bfloat16
    EXP = mybir.ActivationFunctionType.Exp

    nc = bacc.Bacc("TRN2", target_bir_lowering=False, debug=False,
                   num_devices=NCORES)

    xT = nc.dram_tensor("xT", [D, R], f32r, kind="ExternalInput").ap()
    w_qkv = nc.dram_tensor("w_qkv", [D, 3 * HPC * DH], f32r,
                           kind="ExternalInput").ap()
    b_qkv = nc.dram_tensor("b_qkv", [1, 3 * HPC * DH], bf16,
                           kind="ExternalInput").ap()
    w_out = nc.dram_tensor("w_out", [D, D], bf16, kind="ExternalInput").ap()
    b_out = nc.dram_tensor("b_out", [1, D], bf16, kind="ExternalInput").ap()
    tri = nc.dram_tensor("tri", [128, 896], bf16, kind="ExternalInput").ap()
    cst = nc.dram_tensor("cst", [128, 512], bf16, kind="ExternalInput").ap()

    kT_out = nc.dram_tensor("kT_out", [HPC * DH, R], f32,
                            kind="ExternalOutput").ap()
    vT_out = nc.dram_tensor("vT_out", [HPC * DH, R], f32,
                            kind="ExternalOutput").ap()
    o_out = nc.dram_tensor("o_out", [B, W, D], f32,
                           kind="ExternalOutput").ap()

    # per-batch A2A buffers
    a2a_in = [nc.dram_tensor(f"a2a_in{b}", [NCORES * 128, W], bf16)
              for b in range(B)]
    a2a_out = [nc.dram_tensor(f"a2a_out{b}", [NCORES * 128, W], bf16)
               for b in range(B)]
    RG = [list(range(NCORES))]

    with tile.TileContext(nc) as tc, ExitStack() as ctx:
        persist = ctx.enter_context(tc.tile_pool(name="persist", bufs=1))
        qT = persist.tile([128, R], bf16)
        kT = persist.tile([128, R], f32)
        kT16 = persist.tile([128, R], bf16)
        vA = persist.tile([128, R // 128, HPC, 2 * DH], bf16)
        triE = persist.tile([128, 896], bf16)
        ident = persist.tile([128, 128], f32)
        ones_sb = persist.tile([128, 512], bf16)
        bout_sb = persist.tile([1, D], bf16)

        make_identity(nc, ident)
        nc.sync.dma_start(out=ones_sb, in_=cst)
        nc.sync.dma_start(out=triE, in_=tri)
        nc.sync.dma_start(out=bout_sb, in_=b_out)
        # fill vA with ones; phase-1 v copies overwrite cols [0, DH).
        # cols [DH, 2*DH) stay 1.0 so the AV matmul replicates sum(exp)
        # onto 64 psum partitions (normalizer broadcast for free).
        nc.vector.memset(vA[:, :, :, :], 1.0)

        # Pools (SBUF) — all phases coexist for interleaved emission.
        p1 = ctx.enter_context(tc.tile_pool(name="p1", bufs=2))
        p1w = ctx.enter_context(tc.tile_pool(name="p1w", bufs=1))
        p2 = ctx.enter_context(tc.tile_pool(name="p2", bufs=3))
        p2s = ctx.enter_context(tc.tile_pool(name="p2s", bufs=2))
        p3 = ctx.enter_context(tc.tile_pool(name="p3", bufs=3))
        p3w = ctx.enter_context(tc.tile_pool(name="p3w", bufs=1))
        p3o = ctx.enter_context(tc.tile_pool(name="p3o", bufs=2))
        ps2s = ctx.enter_context(tc.tile_pool(name="ps2s", bufs=2,
                                              space="PSUM"))
        ps2c = ctx.enter_context(tc.tile_pool(name="ps2c", bufs=1,
                                              space="PSUM"))
        ps1cm = tc.tile_pool(name="ps1", bufs=2, space="PSUM")
        ps1 = ps1cm.__enter__()

        wq = p1w.tile([128, NKD, 3 * HPC * DH], f32r)
        nc.sync.dma_start(out=wq,
                          in_=w_qkv.rearrange("(kt p) c -> p kt c", p=128))
        bq = p1w.tile([1, 3 * HPC * DH], bf16)
        nc.sync.dma_start(out=bq, in_=b_qkv)
        wo = p3w.tile([128, NKD, D], bf16)
        nc.sync.dma_start(out=wo,
                          in_=w_out.rearrange("(kt p) c -> p kt c", p=128))

        def p1_chunk(rc):
            """Generator: phase-1 work for rows chunk rc, in small quanta so
            it can fill PE gaps inside the exp-paced attention stream."""
            r0 = rc * RC
            xt = p1.tile([128, NKD, RC], f32r, tag="xt")
            nc.sync.dma_start(
                out=xt,
                in_=xT[:, r0:r0 + RC].rearrange("(kt p) r -> p kt r", p=128))
            yield
            for chain, c0 in (("q", 0), ("k", 128), ("v", 256)):
                ps_t = ps1.tile([128, RC], f32, tag="p1ps")
                nc.tensor.matmul(out=ps_t[:, :], lhsT=bq[0:1, c0:c0 + 128],
                                 rhs=ones_sb[0:1, 0:RC],
                                 start=True, stop=False)
                for kt in range(NKD):
                    nc.tensor.matmul(out=ps_t[:, :],
                                     lhsT=wq[:, kt, c0:c0 + 128],
                                     rhs=xt[:, kt, :],
                                     start=False, stop=(kt == NKD - 1))
                    if kt % 3 == 2:
                        yield
                if chain == "q":
                    nc.vector.tensor_copy(out=qT[:, r0:r0 + RC],
                                          in_=ps_t[:, :])
                elif chain == "k":
                    nc.vector.tensor_copy(out=kT16[:, r0:r0 + RC],
                                          in_=ps_t[:, :])
                    kt_sb = p1.tile([128, RC], f32, tag="ktsb")
                    nc.vector.tensor_copy(out=kt_sb[:, :], in_=ps_t[:, :])
                    nc.sync.dma_start(out=kT_out[:, r0:r0 + RC],
                                      in_=kt_sb[:, :])
                else:
                    vt_sb = p1.tile([128, RC], f32, tag="vtsb")
                    nc.vector.tensor_copy(out=vt_sb[:, :], in_=ps_t[:, :])
                    nc.sync.dma_start(out=vT_out[:, r0:r0 + RC],
                                      in_=vt_sb[:, :])
                yield
            for t4 in range(RC // 128):
                ps_vt = ps1.tile([128, 128], f32, tag="p1ps")
                nc.tensor.transpose(ps_vt[:, :],
                                    vt_sb[:, t4 * 128:(t4 + 1) * 128], ident)
                rt = rc * (RC // 128) + t4
                for hh in range(HPC):
                    nc.vector.tensor_copy(
                        out=vA[:, rt, hh, 0:DH],
                        in_=ps_vt[:, hh * DH:(hh + 1) * DH])
                if t4 % 2 == 1:
                    yield

        def proj_chunk(b, ctxf, ps3):
            """Generator: out-projection for batch b (A2A(b) must be done)."""
            with nc.named_scope(f"proj{b}"):
                for rt in range(W // 128):
                    for nch in range(D // 512):
                        ps_o = ps3.tile([128, 512], f32, tag="po")
                        nc.tensor.matmul(
                            out=ps_o[:, :],
                            lhsT=ones_sb[0:1, 0:128],
                            rhs=bout_sb[0:1, nch * 512:(nch + 1) * 512],
                            start=True, stop=False)
                        for kt in range(NKD):
                            nc.tensor.matmul(
                                out=ps_o[:, :],
                                lhsT=ctxf[:, kt, rt * 128:(rt + 1) * 128],
                                rhs=wo[:, kt, nch * 512:(nch + 1) * 512],
                                start=False, stop=(kt == NKD - 1))
                            if kt % 3 == 2:
                                yield
                        ob = p3o.tile([128, 512], f32, tag="ob")
                        nc.vector.tensor_copy(out=ob[:, :], in_=ps_o[:, :])
                        nc.sync.dma_start(
                            out=o_out[b, rt * 128:(rt + 1) * 128,
                                      nch * 512:(nch + 1) * 512],
                            in_=ob[:, :])
                        yield

        _SENT = object()

        def attention(b, filler):
            def pump(n):
                for _ in range(n):
                    if next(filler, _SENT) is _SENT:
                        return

            with nc.named_scope(f"att{b}"):
                for qc in range(NQC):
                    q0 = b * S + qc * QC
                    ctx_ps = ps2c.tile([2 * DH, HPC, QC], f32, tag="ctx")
                    nk = (qc * QC) // 128 + 4
                    jbase = (qc * QC) // 128

                    def av(pend, last):
                        kt, off, pex = pend
                        for s in range(HPC):
                            nc.tensor.matmul(
                                out=ctx_ps[:, s, off:QC],
                                lhsT=vA[:, (b * S) // 128 + kt, s, :],
                                rhs=pex[:, s, off:QC],
                                start=(kt == 0), stop=last)

                    pend = None
                    for kt in range(nk):
                        k0 = b * S + kt * 128
                        j = kt - jbase
                        off = 128 * j if j >= 0 else 0
                        scp = ps2s.tile([128, HPC, QC], f32, tag="sc")
                        # two half-array (K=64) score matmuls run concurrently
                        for s in range(HPC):
                            nc.tensor.matmul(
                                out=scp[:, s, :],
                                lhsT=kT16[DH * s:DH * s + DH, k0:k0 + 128],
                                rhs=qT[DH * s:DH * s + DH, q0:q0 + QC],
                                start=True, stop=True)
                        ex2 = p2.tile([128, HPC, QC], bf16, tag="ex")
                        nc.scalar.activation(out=ex2[:, :, off:QC],
                                             in_=scp[:, :, off:QC], func=EXP)
                        if j >= 0:
                            for s in range(HPC):
                                nc.vector.tensor_mul(
                                    ex2[:, s, off:QC], ex2[:, s, off:QC],
                                    triE[:, 384:384 + QC - off])
                        if pend is not None:
                            av(pend, False)
                        pump(2)
                        pend = (kt, off, ex2)
                    av(pend, True)
                    # normalize both heads: the AV matmul already replicated
                    # sum(exp) onto partitions [DH, 2*DH)
                    for s in range(HPC):
                        bc = p2s.tile([DH, QC], f32, tag="bc")
                        nc.vector.reciprocal_approx_fast(
                            out=bc[:, :], in_=ctx_ps[DH:2 * DH, s, :])
                        cx = p2s.tile([DH, QC], bf16, tag="cx")
                        nc.vector.tensor_mul(cx[:, :], ctx_ps[0:DH, s, :],
                                             bc[:, :])
                        rel = qc * QC
                        for t in range(max(1, QC // W)):
                            j_sh = rel // W + t
                            ww = min(W, QC)
                            nc.sync.dma_start(
                                out=a2a_in[b][128 * j_sh + DH * s:
                                              128 * j_sh + DH * s + DH, :],
                                in_=cx[:, t * ww:(t + 1) * ww])
            pump(10 ** 9)

        def run_gen(g):
            for _ in g:
                pass

        import itertools
        NCH = S // RC          # phase-1 chunks per batch
        for i in range(NCH):
            run_gen(p1_chunk(i))
        ctxf_tiles = {}
        ps3 = None
        for b in range(B):
            if b < B - 1:
                filler = itertools.chain(
                    *[p1_chunk(NCH * (b + 1) + i) for i in range(NCH)])
            else:
                ps1cm.__exit__(None, None, None)
                ps3 = ctx.enter_context(
                    tc.tile_pool(name="ps3", bufs=2, space="PSUM"))
                filler = itertools.chain(
                    *[proj_chunk(bb, ctxf_tiles[bb], ps3)
                      for bb in range(B - 1)])
            attention(b, filler)
            nc.gpsimd.collective_compute(
                "AllToAll", mybir.AluOpType.bypass, replica_groups=RG,
                ins=[a2a_in[b][:]], outs=[a2a_out[b][:]])
            ctxf = p3.tile([128, NKD, W], bf16, tag="ctxf")
            nc.sync.dma_start(
                out=ctxf,
                in_=a2a_out[b][:].rearrange("(kt p) r -> p kt r", p=128))
            ctxf_tiles[b] = ctxf
        run_gen(proj_chunk(B - 1, ctxf_tiles[B - 1], ps3))

    nc.compile()
    return nc


def _get_nc():
    if "nc" not in _CACHE:
        _CACHE["nc"] = _build()
    return _CACHE["nc"]


def _host_inputs(x, w_in, b_in, w_out, b_out):
    """Build the 8 per-core input maps."""
    x = np.asarray(x, dtype=np.float32)
    w_in = np.asarray(w_in, dtype=np.float32)
    b_in = np.asarray(b_in, dtype=np.float32)
    w_out = np.asarray(w_out, dtype=np.float32)
    b_out = np.asarray(b_out, dtype=np.float32)
    Bb, Ss, _ = x.shape
    R = Bb * Ss

    xT = np.ascontiguousarray(x.reshape(R, D).T)
    scale = np.float32(1.0 / np.sqrt(DH))

    # causal triangle, extended for the 4 diagonal offsets:
    # triE[kk, u] = 1 iff u >= 384 + kk  (u in [0, 896))
    u = np.arange(896, dtype=np.int32)[None, :]
    kk = np.arange(128, dtype=np.int32)[:, None]
    triE = (u >= 384 + kk).astype(ml_dtypes.bfloat16)

    w_out_c = np.ascontiguousarray(w_out).astype(ml_dtypes.bfloat16)
    b_out_c = b_out.reshape(1, D).astype(ml_dtypes.bfloat16)

    in_maps = []
    for c in range(NCORES):
        cols = slice(HPC * DH * c, HPC * DH * (c + 1))
        wq = w_in[:, 0:D][:, cols] * scale
        wk = w_in[:, D:2 * D][:, cols]
        wv = w_in[:, 2 * D:3 * D][:, cols]
        bq = b_in[0:D][cols] * scale
        bk = b_in[D:2 * D][cols]
        bv = b_in[2 * D:3 * D][cols]
        in_maps.append({
            "xT": xT,
            "w_qkv": np.ascontiguousarray(
                np.concatenate([wq, wk, wv], axis=1)),
            "b_qkv": np.concatenate([bq, bk, bv]).reshape(1, -1)
                .astype(ml_dtypes.bfloat16),
            "w_out": w_out_c,
            "b_out": b_out_c,
            "tri": triE,
            "cst": np.ones((128, 512), dtype=ml_dtypes.bfloat16),
        })
    return in_maps


def _assemble(results, B=4, S=2048):
    R = B * S
    W = S // NCORES
    k = np.empty((B, H, S, DH), dtype=np.float32)
    v = np.empty((B, H, S, DH), dtype=np.float32)
    out_flat = np.empty((R, D), dtype=np.float32)
    for c in range(NCORES):
        kc = results[c]["kT_out"].reshape(HPC, DH, B, S)
        vc = results[c]["vT_out"].reshape(HPC, DH, B, S)
        k[:, HPC * c:HPC * (c + 1)] = kc.transpose(2, 0, 3, 1)
        v[:, HPC * c:HPC * (c + 1)] = vc.transpose(2, 0, 3, 1)
        oc = results[c]["o_out"]          # [B, W, D]
        for b in range(B):
            g0 = b * S + W * c
            out_flat[g0:g0 + W] = oc[b]
    out = out_flat.reshape(B, S, D)
    return out, (k, v)


def kernel(x, w_in, b_in, w_out, b_out):
    from concourse.bass_utils import run_bass_kernel_spmd
    nc = _get_nc()
    in_maps = _host_inputs(x, w_in, b_in, w_out, b_out)
    res = run_bass_kernel_spmd(nc, in_maps, list(range(NCORES)))
    return _assemble(res.results)
